# revision 47
# baseline (speedup 1.0000x reference)
"""Modulated Conv2D (StyleGAN2-style) Trainium2 Bass kernel.

Problem shapes (hardcoded):
  x: [16, 256, 64, 64] f32    y: [16, 512] f32
  weights: [256, 256, 3, 3]   bias: [256]
  style_w: [256, 512]         style_b: [256]
  out: [16, 256, 64, 64] f32

Formulation: fold the per-sample style modulation into the weights
(classic StyleGAN2), so x needs no per-pixel scaling at all:
  style[b,i] = y[b] @ style_w[i] + style_b[i]      (tiny PE matmul)
  w_mod[b][i,kk,o] = wT[i,kk,o] * style[b,i]       (DVE)
  out[b,o] = (1/wstd[b,o]) * conv(x[b], w_mod[b])[o] + bias[o]
  wstd[b,o] = sqrt(sum_{i,kk} wT[i,kk,o]^2 * style[b,i]^2 + eps)

Host-side layout prep (pure data movement / dtype packing):
  - wT = weights transposed to [Cin, kk, Cout], bf16: no PE transposes,
    contiguous lhsT slices.
  - aux = one packed bf16 tile carrying style_w.T, y.T, style_b (as a
    rank-1 extra contraction plane), bias and eps. DMAs here cost
    ~17ns/partition-descriptor (~2.2us per 128-partition transfer), so
    everything small rides one descriptor set.
  - x zero-padded to [66,66], bf16: DMA lands matmul-ready tiles.
  - device output bf16, upcast on host (budget 2e-2, bf16 out ~2e-3).

Sharding: data-parallel over batch, 2 samples per core across 8 cores.
Conv = 9 shifted matmuls per Cin-block (x2) accumulating in PSUM f32,
8-row x 64-col chunks (512-elem free dim = one PSUM bank).

PSUM: 8 single-buffer banks; conv chunks roll pa0..pa7, style borrows
pa4/pa5 and sigma pa6/pa7 (drain early, so the second conv block never
waits on the winv-gated drains of the first).
"""

import numpy as np
import ml_dtypes

import concourse.bass as bass
import concourse.tile as tile
from concourse import bacc, mybir
from concourse import bass_utils

EPS = 1e-8
P = 128
B_LOC = 2          # samples per core
B_FULL = 16
CIN, COUT = 256, 256
NI, NO = CIN // P, COUT // P   # 2, 2
S = 512
NS = S // P        # 4 style contraction blocks
KK = 9             # 3x3 taps
H = W = 64
HP, WP = H + 2, W + 2  # zero-padded image
N_CORES = 8

# aux packing offsets (bf16 columns)
A_IT0 = 0                      # NS+1 planes of [128 swt cols + 2 y cols]
A_W0 = P + B_LOC               # plane width for it0 block
A_IT1 = (NS + 1) * A_W0        # NS+1 planes of [128 swt cols]
A_BIAS = A_IT1 + (NS + 1) * P  # 2 bias cols
A_EPS = A_BIAS + NO            # eps col
A_TOT = A_EPS + 1

F32 = mybir.dt.float32
BF16 = mybir.dt.bfloat16
AF = mybir.ActivationFunctionType


def _chain(instrs, reason):
    """Force program order on one engine (guides the tile scheduler)."""
    for a, b in zip(instrs[1:], instrs[:-1]):
        bass._add_dep_helper(a.ins, b.ins, sync=False, reason=reason)


def build_conv2dmod(nc):
    xp = nc.dram_tensor("xp", [B_LOC, CIN, HP, WP], BF16, kind="ExternalInput")
    wt = nc.dram_tensor("wt", [CIN, KK, COUT], BF16, kind="ExternalInput")
    aux = nc.dram_tensor("aux", [P, A_TOT], BF16, kind="ExternalInput")
    out = nc.dram_tensor("out", [B_LOC, COUT, H, W], BF16, kind="ExternalOutput")

    with tile.TileContext(nc) as tc:
        with (
            tc.tile_pool(name="consts", bufs=1) as consts,
            tc.tile_pool(name="temps", bufs=1) as temps,
            tc.tile_pool(name="wmod_pool", bufs=2) as wmod_pool,
            tc.tile_pool(name="xs_pool", bufs=1) as xs_pool,
            tc.tile_pool(name="out_pool", bufs=3) as out_pool,
            tc.tile_pool(name="psum", bufs=1, space="PSUM") as psum,
        ):
            # ---------------- tiles ----------------
            aux_t = consts.tile([P, A_TOT], BF16)
            wt_t = [consts.tile([P, KK, COUT], BF16, name=f"wt{i}", tag=f"wt{i}")
                    for i in range(NI)]
            xs = {}
            for b in range(B_LOC):
                for it in range(NI):
                    xs[(b, it)] = xs_pool.tile([P, HP, WP], BF16,
                                               name=f"xs{b}_{it}", tag=f"xs{b}_{it}")

            # bias/eps upconverted once to f32 (engines require f32 scalars)
            cst_t = consts.tile([P, NO + 1], F32)

            def bias_ap(ot):
                return cst_t[:, ot:ot + 1]

            eps_ap = cst_t[:, NO:NO + 1]

            # pre-warm the ACT function table that Sqrt lives in;
            # dependency-free so its async table load fires immediately
            warm_src = consts.tile([P, 1], F32)
            nc.gpsimd.memset(warm_src[:], EPS)
            lafs_warm = consts.tile([P, 1], F32)
            warm_i = nc.scalar.activation(lafs_warm[:], warm_src[:], AF.Sqrt)

            # ------------- DMA rings, ordered by when they gate compute -------
            # scalar HW ring: aux (style path) then the transposed weights
            scalar_ring = [
                nc.scalar.dma_start(aux_t[:], aux.ap()),
                nc.scalar.dma_start(wt_t[0][:], wt.ap()[0:P]),
                nc.scalar.dma_start(wt_t[1][:], wt.ap()[P:2 * P]),
            ]
            _chain([warm_i] + scalar_ring, "warm then scalar ring order")
            nc.gpsimd.tensor_copy(cst_t[:], aux_t[:, A_BIAS:A_TOT])

            # sync HW ring: x tiles; x01 staggers behind wt0 so the head's
            # critical weight DMA keeps full HBM bandwidth
            def load_x(b, it, after=None):
                i = nc.sync.dma_start(xs[(b, it)][:],
                                      xp.ap()[b, it * P:(it + 1) * P])
                if after is not None:
                    bass._add_dep_helper(i.ins, after.ins, sync=True,
                                         reason="stagger x behind weights")
                return i

            load_x(0, 0)
            load_x(0, 1, after=scalar_ring[1])

            # ---------- style (PE): [P(cin), B_LOC] per cin block ----------
            def sw_lhs(it, sb):
                base = (A_IT0 + sb * A_W0) if it == 0 else (A_IT1 + sb * P)
                return aux_t[:, base:base + P]

            def sw_lhs1(it):
                base = (A_IT0 + NS * A_W0) if it == 0 else (A_IT1 + NS * P)
                return aux_t[0:1, base:base + P]

            y_rhs = [aux_t[:, A_IT0 + sb * A_W0 + P:A_IT0 + sb * A_W0 + P + B_LOC]
                     for sb in range(NS + 1)]

            style_col = []
            style2 = []
            style_sq_i = []
            for it in range(NI):
                ps = psum.tile([P, B_LOC], F32, name=f"styp{it}", tag=f"pa{4 + it}")
                for sb in range(NS):
                    nc.tensor.matmul(ps[:], sw_lhs(it, sb), y_rhs[sb][:],
                                     start=(sb == 0), stop=False)
                # style_b rides as a rank-1 contraction on partition 0
                nc.tensor.matmul(ps[:], sw_lhs1(it), y_rhs[NS][0:1, :],
                                 start=False, stop=True)
                sc = consts.tile([P, B_LOC], F32, name=f"stc{it}", tag=f"stc{it}")
                nc.scalar.activation(sc[:], ps[:], AF.Identity)
                s2 = consts.tile([P, B_LOC], F32, name=f"st2{it}", tag=f"st2{it}")
                style_sq_i.append(nc.vector.tensor_mul(s2[:], sc[:], sc[:]))
                style_col.append(sc)
                style2.append(s2)

            # ---------- per-sample modulated weights (DVE) ----------
            w_mod = {}
            wmod_i = []

            def make_wmod(b, it):
                t = wmod_pool.tile([P, KK, COUT], BF16, name=f"wm{b}_{it}",
                                   tag=f"wm{it}")
                wmod_i.append(nc.vector.tensor_scalar_mul(
                    t[:], wt_t[it][:], style_col[it][:, b:b + 1]))
                w_mod[(b, it)] = t

            make_wmod(0, 0)
            make_wmod(0, 1)
            make_wmod(1, 0)
            make_wmod(1, 1)
            # DVE order: sq0, wm00, sq1, then the remaining wmods
            _chain([style_sq_i[0], wmod_i[0], style_sq_i[1]] + wmod_i[1:],
                   "style squares then wmods in order")

            # ---------- main conv block: 18*nchunks matmuls per call ----------
            bank_ptr = [0]

            def mm_block(b, ot, r0, nchunks):
                pcs = []
                for c in range(nchunks):
                    pcs.append(psum.tile([P, 8, W], F32, name=f"pc{b}{ot}{r0}_{c}",
                                         tag=f"pa{bank_ptr[0] % 8}"))
                    bank_ptr[0] += 1
                first, last = (0, 0), (NI - 1, KK - 1)
                for it in range(NI):
                    for kk in range(KK):
                        dy, dx = kk // 3, kk % 3
                        lhsT = w_mod[(b, it)][:, kk, ot * P:(ot + 1) * P]
                        for c in range(nchunks):
                            rs = r0 + c * 8 + dy
                            nc.tensor.matmul(
                                pcs[c][:], lhsT, xs[(b, it)][:, rs:rs + 8, dx:dx + W],
                                start=((it, kk) == first), stop=((it, kk) == last),
                            )
                return pcs

            def out_block(b, ot, r0, pcs, dma_rows=32, engine="scalar"):
                n = len(pcs)
                oh = out_pool.tile([P, 8 * n, W], BF16, name=f"oh{b}{ot}{r0}",
                                   tag="oh")
                done = 0
                for c in range(n):
                    if engine == "scalar":
                        nc.scalar.activation(
                            oh[:, c * 8:(c + 1) * 8, :], pcs[c][:], AF.Identity,
                            bias=bias_ap(ot), scale=winv[ot][:, b:b + 1],
                        )
                    else:
                        nc.vector.tensor_scalar(
                            oh[:, c * 8:(c + 1) * 8, :], pcs[c][:],
                            winv[ot][:, b:b + 1], bias_ap(ot),
                            mybir.AluOpType.mult, mybir.AluOpType.add,
                        )
                    rows = (c + 1) * 8
                    if rows - done >= dma_rows or c == n - 1:
                        nc.sync.dma_start(
                            out.ap()[b, ot * P:(ot + 1) * P, r0 + done:r0 + rows, :],
                            oh[:, done:rows, :])
                        done = rows

            # first conv block for sample 0 goes as early as possible
            pcs_h0 = mm_block(0, 0, 0, 4)

            # ---------- demod path (behind the first block) ----------
            # W2T[i_part, o] = sum_kk wT[i,kk,o]^2; GPSIMD squares into a
            # [p, o, kk] layout so the DVE reduce reads contiguously
            w2t = [consts.tile([P, COUT], F32, name=f"w2t{i}", tag=f"w2t{i}")
                   for i in range(NI)]
            sqs = [temps.tile([P, COUT, KK], F32, name=f"sq{i}", tag=f"sq{i}")
                   for i in range(NI)]
            red_i = []
            for it in range(NI):
                nc.gpsimd.tensor_mul(sqs[it][:].rearrange("p o kk -> p kk o"),
                                     wt_t[it][:], wt_t[it][:])
                red_i.append(nc.vector.reduce_sum(w2t[it][:], sqs[it][:],
                                                  axis=mybir.AxisListType.X))
            _chain([wmod_i[-1]] + red_i, "w2 reduces after wmods")

            # sigma[o_part, b] = sum_i W2T[i,o] * style2[i,b]  (PE, f32)
            winv = []
            for ot in range(NO):
                ps = psum.tile([P, B_LOC], F32, name=f"sig{ot}", tag=f"pa{6 + ot}")
                for it in range(NI):
                    nc.tensor.matmul(
                        ps[:], w2t[it][:, ot * P:(ot + 1) * P], style2[it][:],
                        start=(it == 0), stop=(it == NI - 1),
                    )
                wstd = consts.tile([P, B_LOC], F32, name=f"wstd{ot}", tag=f"wstd{ot}")
                nc.scalar.activation(wstd[:], ps[:], AF.Sqrt, bias=eps_ap)
                wi = consts.tile([P, B_LOC], F32, name=f"winv{ot}", tag=f"winv{ot}")
                nc.vector.reciprocal(wi[:], wstd[:])
                winv.append(wi)

            # ---------- rest of the schedule ----------
            out_block(0, 0, 0, pcs_h0)
            # sample-1 x loads queue on the sync ring behind the first out
            # stores, well clear of the head's critical bandwidth
            load_x(1, 0)
            load_x(1, 1)
            out_block(0, 0, 32, mm_block(0, 0, 32, 4))
            for half in range(2):
                out_block(0, 1, half * 32, mm_block(0, 1, half * 32, 4))
            for half in range(2):
                out_block(1, 0, half * 32, mm_block(1, 0, half * 32, 4))
            out_block(1, 1, 0, mm_block(1, 1, 0, 4))
            # shrinking tail; the final drain runs on the (idle) vector
            # engine, which drains PSUM faster than ACT
            out_block(1, 1, 32, mm_block(1, 1, 32, 2))
            out_block(1, 1, 48, mm_block(1, 1, 48, 1))
            out_block(1, 1, 56, mm_block(1, 1, 56, 1), engine="vector")
    return nc


_CACHED_NC = None


def _get_nc():
    global _CACHED_NC
    if _CACHED_NC is None:
        nc = bacc.Bacc("TRN2", target_bir_lowering=False, debug=False,
                       num_devices=N_CORES)
        build_conv2dmod(nc)
        nc.compile()
        _CACHED_NC = nc
    return _CACHED_NC


def kernel(x, y, weights, bias, style_w, style_b, _trace=False):
    x = np.asarray(x, dtype=np.float32)
    y = np.asarray(y, dtype=np.float32)
    weights = np.asarray(weights, dtype=np.float32)
    bias = np.asarray(bias, dtype=np.float32)
    style_w = np.asarray(style_w, dtype=np.float32)
    style_b = np.asarray(style_b, dtype=np.float32)

    # host-side layout packing (see module docstring)
    wt = np.ascontiguousarray(
        weights.transpose(1, 2, 3, 0).reshape(CIN, KK, COUT)
    ).astype(ml_dtypes.bfloat16)
    swtf = style_w.T.reshape(NS, P, CIN).transpose(1, 0, 2)  # [sp, sb, i]
    ytf = y.T.reshape(NS, P, B_FULL).transpose(1, 0, 2)      # [sp, sb, b_full]
    xp = np.zeros((B_FULL, CIN, HP, WP), dtype=ml_dtypes.bfloat16)
    xp[:, :, 1:H + 1, 1:W + 1] = x.astype(ml_dtypes.bfloat16)

    aux_base = np.zeros((P, A_TOT), dtype=ml_dtypes.bfloat16)
    a0 = aux_base[:, A_IT0:A_IT1].reshape(P, NS + 1, A_W0)
    a0[:, :NS, :P] = swtf[:, :, 0:P]
    a0[0, NS, :P] = style_b[0:P].astype(ml_dtypes.bfloat16)
    a0[0, NS, P:] = 1.0
    a1 = aux_base[:, A_IT1:A_BIAS].reshape(P, NS + 1, P)
    a1[:, :NS, :] = swtf[:, :, P:2 * P]
    a1[0, NS, :] = style_b[P:2 * P].astype(ml_dtypes.bfloat16)
    aux_base[:, A_BIAS:A_BIAS + NO] = bias.reshape(NO, P).T
    aux_base[:, A_EPS] = EPS

    nc = _get_nc()
    in_maps = []
    for c in range(N_CORES):
        aux_c = aux_base.copy()
        aux_c[:, A_IT0:A_IT1].reshape(P, NS + 1, A_W0)[:, :NS, P:] = \
            ytf[:, :, c * B_LOC:(c + 1) * B_LOC]
        in_maps.append({
            "xp": np.ascontiguousarray(xp[c * B_LOC:(c + 1) * B_LOC]),
            "wt": wt,
            "aux": aux_c,
        })
    res = bass_utils.run_bass_kernel_spmd(
        nc, in_maps, core_ids=list(range(N_CORES)), trace=_trace
    )
    out = np.concatenate([r["out"] for r in res.results], axis=0).astype(np.float32)
    if _trace:
        kernel.last_results = res
    return out
